# revision 5
# baseline (speedup 1.0000x reference)
"""Trainium2 Bass kernel for nn_MultiHeadAttention_65017214927599.

Fused MHA + residual + LayerNorm, sharded over 8 NeuronCores as
(batch, query-half): core c -> batch c//2, queries [ (c%2)*1024, +1024 ).
Each core computes all 16 heads for its 1024 queries and recomputes the
K/V projections for its batch (no cross-core communication).

Device-side layouts are feature-major ("transposed"): scores are computed
as scores^T [k, q] so the PV matmul can consume the (unnormalized,
masked) probabilities directly as its moving operand, and an ones-column
appended to V makes the same matmul emit the softmax row-sum Z for free.
Probabilities leave the device unnormalized in bf16 together with the
per-row reciprocal; the host fuses the normalization into the transpose
it has to do anyway.
"""

import numpy as np

B, S, D, H = 4, 2048, 1024, 16
DH = D // H  # 64
NCORES = 8
SQ = S // 2  # queries per core = 1024
QBLK = 512
NQB = SQ // QBLK  # 2
KC = S // 128  # 16 k-chunks
DC = D // 128  # 8 contraction chunks
FC = D // 128  # 8 feature chunks
LN_EPS = 1e-12
SM_SCALE = 1.0 / np.sqrt(np.float32(DH))  # 0.125

_CACHE = {}


def _build_nc():
    import concourse.tile as tile
    from concourse import bacc, mybir

    f32 = mybir.dt.float32
    bf16 = mybir.dt.bfloat16
    Alu = mybir.AluOpType
    Act = mybir.ActivationFunctionType
    X = mybir.AxisListType.X

    nc = bacc.Bacc()

    # ---- DRAM I/O ----
    qT = nc.dram_tensor("qT", [D, SQ], bf16, kind="ExternalInput")
    kT = nc.dram_tensor("kT", [D, S], bf16, kind="ExternalInput")
    vT = nc.dram_tensor("vT", [D, S], bf16, kind="ExternalInput")
    mk = nc.dram_tensor("mk", [S, SQ], bf16, kind="ExternalInput")
    resid = nc.dram_tensor("resid", [SQ, D], f32, kind="ExternalInput")
    wqT = nc.dram_tensor("wqT", [D, D], bf16, kind="ExternalInput")
    wvT = nc.dram_tensor("wvT", [D, D], bf16, kind="ExternalInput")
    woT = nc.dram_tensor("woT", [64, H, D], bf16, kind="ExternalInput")
    bqpp = nc.dram_tensor("bqpp", [128, FC], f32, kind="ExternalInput")
    bvf = nc.dram_tensor("bvf", [128, D], f32, kind="ExternalInput")
    gammaf = nc.dram_tensor("gammaf", [128, D], f32, kind="ExternalInput")
    betaf = nc.dram_tensor("betaf", [128, D], f32, kind="ExternalInput")

    probsT = nc.dram_tensor("probsT", [H, S, SQ], bf16, kind="ExternalOutput")
    recout = nc.dram_tensor("recout", [H, NQB, QBLK], f32, kind="ExternalOutput")
    outn = nc.dram_tensor("outn", [SQ, D], f32, kind="ExternalOutput")

    with tile.TileContext(nc) as tc:
        with tc.tile_pool(name="p_ctx", bufs=1) as p_ctx:
            # ctx^T, head-major on 64 partitions: [64, head, q] (normalized)
            ctx = p_ctx.tile([64, H, SQ], bf16, name="ctx")

            with tc.tile_pool(name="p_proj", bufs=1) as p_proj:
                # q~ / k~ feature-major [f_part, f_chunk, seq]
                qh = p_proj.tile([128, FC, SQ], bf16, name="qh")
                kh = p_proj.tile([128, FC, S], bf16, name="kh")
                # v~ s-major with ones column: [s_part, s_chunk, head, 65]
                vv = p_proj.tile([128, KC, H, DH + 1], bf16, name="vv")
                bq_sb = p_proj.tile([128, FC], f32, name="bq_sb")
                nc.sync.dma_start(out=bq_sb, in_=bqpp[:, :])

                # ---------- Phase A: projections ----------
                with tc.tile_pool(name="p_aw", bufs=1) as p_aw, \
                     tc.tile_pool(name="p_aps", bufs=3, space="PSUM") as p_aps:
                    wq_sb = p_aw.tile([128, DC, D], bf16, name="wq_sb")
                    nc.sync.dma_start(
                        out=wq_sb, in_=wqT[:, :].rearrange("(c p) f -> p c f", p=128))
                    wv_sb = p_aw.tile([128, DC, D], bf16, name="wv_sb")
                    nc.sync.dma_start(
                        out=wv_sb, in_=wvT[:, :].rearrange("(c p) f -> p c f", p=128))
                    bv_sb = p_aw.tile([128, D], f32, name="bv_sb")
                    nc.sync.dma_start(out=bv_sb, in_=bvf[:, :])

                    # q~ = (Wq @ query^T): [f, q]
                    with tc.tile_pool(name="p_ain1", bufs=1) as p_ain:
                        qT_sb = p_ain.tile([128, DC, SQ], bf16, name="qT_sb")
                        nc.sync.dma_start(
                            out=qT_sb,
                            in_=qT[:, :].rearrange("(c p) q -> p c q", p=128))
                        for fc in range(FC):
                            for qb in range(SQ // 512):
                                ps = p_aps.tile([128, 512], f32, name="ps_a",
                                                tag="ps_a")
                                for dc in range(DC):
                                    nc.tensor.matmul(
                                        ps,
                                        lhsT=wq_sb[:, dc, fc * 128:(fc + 1) * 128],
                                        rhs=qT_sb[:, dc, qb * 512:(qb + 1) * 512],
                                        start=(dc == 0), stop=(dc == DC - 1))
                                nc.scalar.activation(
                                    qh[:, fc, qb * 512:(qb + 1) * 512], ps,
                                    Act.Identity, bias=bq_sb[:, fc:fc + 1])

                    # k~ = (Wq @ key^T)  (source bug: Wq/bq applied to key)
                    with tc.tile_pool(name="p_ain2", bufs=1) as p_ain:
                        kT_sb = p_ain.tile([128, DC, S], bf16, name="kT_sb")
                        nc.sync.dma_start(
                            out=kT_sb,
                            in_=kT[:, :].rearrange("(c p) s -> p c s", p=128))
                        for fc in range(FC):
                            for sb in range(S // 512):
                                ps = p_aps.tile([128, 512], f32, name="ps_a",
                                                tag="ps_a")
                                for dc in range(DC):
                                    nc.tensor.matmul(
                                        ps,
                                        lhsT=wq_sb[:, dc, fc * 128:(fc + 1) * 128],
                                        rhs=kT_sb[:, dc, sb * 512:(sb + 1) * 512],
                                        start=(dc == 0), stop=(dc == DC - 1))
                                nc.scalar.activation(
                                    kh[:, fc, sb * 512:(sb + 1) * 512], ps,
                                    Act.Identity, bias=bq_sb[:, fc:fc + 1])

                    # v~ = value @ Wv.T (s-major) + bv, interleaved with ones col
                    with tc.tile_pool(name="p_ain3", bufs=1) as p_ain:
                        vT_sb = p_ain.tile([128, DC, S], bf16, name="vT_sb")
                        nc.sync.dma_start(
                            out=vT_sb,
                            in_=vT[:, :].rearrange("(c p) s -> p c s", p=128))
                        for sc in range(KC):
                            for fb in range(D // 512):
                                ps = p_aps.tile([128, 512], f32, name="ps_a",
                                                tag="ps_a")
                                for dc in range(DC):
                                    nc.tensor.matmul(
                                        ps,
                                        lhsT=vT_sb[:, dc, sc * 128:(sc + 1) * 128],
                                        rhs=wv_sb[:, dc, fb * 512:(fb + 1) * 512],
                                        start=(dc == 0), stop=(dc == DC - 1))
                                nc.vector.tensor_tensor(
                                    vv[:, sc, fb * 8:(fb + 1) * 8, 0:DH],
                                    ps.rearrange("p (h e) -> p h e", e=DH),
                                    bv_sb[:, fb * 512:(fb + 1) * 512].rearrange(
                                        "p (h e) -> p h e", e=DH),
                                    Alu.add)
                    nc.vector.memset(vv[:, :, :, DH:DH + 1], 1.0)

                # ---------- Phase B: attention ----------
                with tc.tile_pool(name="p_bm", bufs=1) as p_bm, \
                     tc.tile_pool(name="p_bun", bufs=1) as p_bun, \
                     tc.tile_pool(name="p_bst", bufs=3) as p_bst, \
                     tc.tile_pool(name="p_bps", bufs=4, space="PSUM") as p_bps, \
                     tc.tile_pool(name="p_bpc", bufs=4, space="PSUM") as p_bpc:
                    mk_sb = p_bm.tile([128, KC, SQ], bf16, name="mk_sb")
                    nc.sync.dma_start(
                        out=mk_sb, in_=mk[:, :].rearrange("(c p) q -> p c q", p=128))

                    for pair in range(H // 2):
                        for qb in range(NQB):
                            un = p_bun.tile([128, KC, 2, QBLK], bf16, name="un",
                                            tag="un")
                            psc = [
                                p_bpc.tile([DH + 1, QBLK], f32, name="psctx",
                                           tag="psctx")
                                for _ in range(2)
                            ]
                            for kc in range(KC):
                                for hh in range(2):
                                    h = 2 * pair + hh
                                    half, fc = h % 2, h // 2
                                    ps_s = p_bps.tile([128, QBLK], f32,
                                                      name="ps_s", tag="ps_s")
                                    nc.tensor.matmul(
                                        ps_s,
                                        lhsT=kh[half * 64:(half + 1) * 64, fc,
                                                kc * 128:(kc + 1) * 128],
                                        rhs=qh[half * 64:(half + 1) * 64, fc,
                                               qb * QBLK:(qb + 1) * QBLK],
                                        start=True, stop=True)
                                    unv = un[:, kc, hh, :]
                                    # unnorm = exp(scores/8) * keepmask
                                    nc.scalar.activation(unv, ps_s, Act.Exp,
                                                         scale=float(SM_SCALE))
                                    nc.vector.tensor_tensor(
                                        unv, unv,
                                        mk_sb[:, kc, qb * QBLK:(qb + 1) * QBLK],
                                        Alu.mult)
                                    nc.tensor.matmul(
                                        psc[hh],
                                        lhsT=vv[:, kc, h, :],
                                        rhs=unv,
                                        start=(kc == 0), stop=(kc == KC - 1))
                            for hh in range(2):
                                h = 2 * pair + hh
                                # unnormalized probs^T out (bf16)
                                nc.sync.dma_start(
                                    out=probsT[h].rearrange(
                                        "(c p) q -> p c q", p=128)[
                                        :, :, qb * QBLK:(qb + 1) * QBLK],
                                    in_=un[:, :, hh, :])
                                # Z -> rec = 1/Z via exp(-ln Z) on ACT
                                lnz = p_bst.tile([128, QBLK], f32, name="lnz",
                                                 tag="lnz")
                                nc.scalar.activation(lnz[64:65, :],
                                                     psc[hh][64:65, :], Act.Ln)
                                rec = p_bst.tile([128, QBLK], f32, name="rec",
                                                 tag="rec")
                                nc.scalar.activation(rec[64:65, :], lnz[64:65, :],
                                                     Act.Exp, scale=-1.0)
                                nc.sync.dma_start(out=recout[h, qb:qb + 1, :],
                                                  in_=rec[64:65, :])
                                # broadcast rec down 64 partitions:
                                # move to partition 0, then gpsimd broadcast
                                rec0 = p_bst.tile([1, QBLK], f32, name="rec0",
                                                  tag="rec0")
                                nc.sync.dma_start(out=rec0, in_=rec[64:65, :])
                                recb = p_bst.tile([64, QBLK], f32, name="recb",
                                                  tag="recb")
                                nc.gpsimd.partition_broadcast(recb, rec0,
                                                              channels=64)
                                # ctx = psum_ctx * rec  (normalized), bf16
                                nc.vector.tensor_tensor(
                                    ctx[:, h, qb * QBLK:(qb + 1) * QBLK],
                                    psc[hh][0:64, :], recb, Alu.mult)

            # ---------- Phase C: output proj + residual + LayerNorm ----------
            with tc.tile_pool(name="p_c", bufs=1) as p_c, \
                 tc.tile_pool(name="p_cst", bufs=2) as p_cst, \
                 tc.tile_pool(name="p_csc", bufs=2) as p_csc, \
                 tc.tile_pool(name="p_cps", bufs=3, space="PSUM") as p_cps:
                wo_sb = p_c.tile([64, H, D], bf16, name="wo_sb")
                nc.sync.dma_start(out=wo_sb, in_=woT[:, :, :])
                re_sb = p_c.tile([128, SQ // 128, D], f32, name="re_sb")
                nc.sync.dma_start(
                    out=re_sb, in_=resid[:, :].rearrange("(c p) f -> p c f", p=128))
                ga_sb = p_c.tile([128, D], f32, name="ga_sb")
                nc.sync.dma_start(out=ga_sb, in_=gammaf[:, :])
                be_sb = p_c.tile([128, D], f32, name="be_sb")
                nc.sync.dma_start(out=be_sb, in_=betaf[:, :])
                xn = p_c.tile([128, SQ // 128, D], f32, name="xn")

                # x = ctx @ Wo.T + (resid + bo):  natural [q, f] orientation
                for qc in range(SQ // 128):
                    for fb in range(D // 512):
                        ps = p_cps.tile([128, 512], f32, name="ps_c", tag="ps_c")
                        for t in range(H):
                            nc.tensor.matmul(
                                ps,
                                lhsT=ctx[:, t, qc * 128:(qc + 1) * 128],
                                rhs=wo_sb[:, t, fb * 512:(fb + 1) * 512],
                                start=(t == 0), stop=(t == H - 1))
                        nc.vector.tensor_tensor(
                            xn[:, qc, fb * 512:(fb + 1) * 512], ps,
                            re_sb[:, qc, fb * 512:(fb + 1) * 512], Alu.add)

                # LayerNorm over f (free dim; per-token stats are per-partition)
                for qc in range(SQ // 128):
                    xq = xn[:, qc, :]
                    sx = p_csc.tile([128, 1], f32, name="sx", tag="sx")
                    nc.vector.tensor_reduce(sx, xq, axis=X, op=Alu.add)
                    xsq = p_cst.tile([128, D], f32, name="xsq", tag="xsq")
                    nc.scalar.activation(xsq, xq, Act.Square)
                    sq = p_csc.tile([128, 1], f32, name="sq", tag="sq")
                    nc.vector.tensor_reduce(sq, xsq, axis=X, op=Alu.add)
                    mean = p_csc.tile([128, 1], f32, name="mean", tag="mean")
                    nc.vector.tensor_scalar_mul(mean, sx, 1.0 / D)
                    msq = p_csc.tile([128, 1], f32, name="msq", tag="msq")
                    nc.vector.tensor_scalar_mul(msq, sq, 1.0 / D)
                    m2 = p_csc.tile([128, 1], f32, name="m2", tag="m2")
                    nc.vector.tensor_mul(m2, mean, mean)
                    m2e = p_csc.tile([128, 1], f32, name="m2e", tag="m2e")
                    nc.vector.tensor_scalar_sub(m2e, m2, LN_EPS)
                    var = p_csc.tile([128, 1], f32, name="var", tag="var")
                    nc.vector.tensor_sub(var, msq, m2e)  # = var + eps
                    std = p_csc.tile([128, 1], f32, name="std", tag="std")
                    nc.scalar.activation(std, var, Act.Sqrt)
                    rstd = p_csc.tile([128, 1], f32, name="rstd", tag="rstd")
                    nc.vector.reciprocal(rstd, std)
                    # out = ((x - mean) * gamma) * rstd + beta
                    t1 = p_cst.tile([128, D], f32, name="t1", tag="t1")
                    nc.vector.scalar_tensor_tensor(
                        t1, in0=xq, scalar=mean, in1=ga_sb,
                        op0=Alu.subtract, op1=Alu.mult)
                    t2 = p_cst.tile([128, D], f32, name="t2", tag="t2")
                    nc.vector.scalar_tensor_tensor(
                        t2, in0=t1, scalar=rstd, in1=be_sb,
                        op0=Alu.mult, op1=Alu.add)
                    nc.sync.dma_start(
                        out=outn[:, :].rearrange("(c p) f -> p c f", p=128)[:, qc, :],
                        in_=t2)

    nc.finalize()
    return nc


def _prep_inputs(query, key, value, attention_mask, Wq, bq, Wv, bv, Wo, bo,
                 gamma, beta):
    import ml_dtypes
    bf16 = ml_dtypes.bfloat16

    WqT = np.ascontiguousarray(Wq.T).astype(bf16)
    WvT = np.ascontiguousarray(Wv.T).astype(bf16)
    # Wo.T [D, D] -> [64, 16, D]: row t*64+p -> [p, t]
    WoTs = np.ascontiguousarray(
        Wo.T.reshape(H, 64, D).transpose(1, 0, 2)).astype(bf16)
    bqpp = np.ascontiguousarray(bq.reshape(FC, 128).T).astype(np.float32)
    bvf = np.ascontiguousarray(np.broadcast_to(bv, (128, D))).astype(np.float32)
    gammaf = np.ascontiguousarray(np.broadcast_to(gamma, (128, D))).astype(
        np.float32)
    betaf = np.ascontiguousarray(np.broadcast_to(beta, (128, D))).astype(
        np.float32)

    in_maps = []
    for c in range(NCORES):
        b, qhalf = c // 2, c % 2
        q0 = qhalf * SQ
        qs = query[b, q0:q0 + SQ]  # [SQ, D]
        in_maps.append({
            "qT": np.ascontiguousarray(qs.T).astype(bf16),
            "kT": np.ascontiguousarray(key[b].T).astype(bf16),
            "vT": np.ascontiguousarray(value[b].T).astype(bf16),
            "mk": np.ascontiguousarray(
                (~attention_mask[b, q0:q0 + SQ, :]).T).astype(bf16),
            "resid": np.ascontiguousarray(qs + bo).astype(np.float32),
            "wqT": WqT,
            "wvT": WvT,
            "woT": WoTs,
            "bqpp": bqpp,
            "bvf": bvf,
            "gammaf": gammaf,
            "betaf": betaf,
        })
    return in_maps


def kernel(**inputs):
    from concourse.bass_utils import run_bass_kernel_spmd

    query = np.asarray(inputs["query"], np.float32)
    key = np.asarray(inputs["key"], np.float32)
    value = np.asarray(inputs["value"], np.float32)
    attention_mask = np.asarray(inputs["attention_mask"], bool)
    Wq = np.asarray(inputs["Wq"], np.float32)
    bq = np.asarray(inputs["bq"], np.float32)
    Wv = np.asarray(inputs["Wv"], np.float32)
    bv = np.asarray(inputs["bv"], np.float32)
    Wo = np.asarray(inputs["Wo"], np.float32)
    bo = np.asarray(inputs["bo"], np.float32)
    gamma = np.asarray(inputs["gamma"], np.float32)
    beta = np.asarray(inputs["beta"], np.float32)

    if "nc" not in _CACHE:
        _CACHE["nc"] = _build_nc()
    nc = _CACHE["nc"]

    in_maps = _prep_inputs(query, key, value, attention_mask, Wq, bq, Wv, bv,
                           Wo, bo, gamma, beta)
    res = run_bass_kernel_spmd(nc, in_maps, core_ids=list(range(NCORES)))
    _CACHE["last_results"] = res

    out = np.empty((B, S, D), np.float32)
    probs = np.empty((B, H, S, S), np.float32)
    for c, r in enumerate(res.results):
        b, qhalf = c // 2, c % 2
        q0 = qhalf * SQ
        out[b, q0:q0 + SQ] = r["outn"]
        rec = r["recout"].reshape(H, SQ)  # [H, SQ]
        unT = r["probsT"]  # [H, S, SQ] bf16
        for h in range(H):
            # probs[q, k] = unT[k, q] * rec[q]
            ph = unT[h].astype(np.float32).T  # [SQ, S]
            ph *= rec[h][:, None]
            probs[b, h, q0:q0 + SQ, :] = ph
    return out, probs
